# revision 11
# baseline (speedup 1.0000x reference)
"""CPMAnt transformer block on 8 TRN2 NeuronCores (Megatron-style TP).

Core c owns 4 attention heads and 1280 FFN columns. Activations are
feature-major (D on partitions). QKV / attention-out / AV / softmax-sum /
sum-of-squares matmuls run in fp8 (e4m3 / e5m2) DoubleRow mode (2 k-tiles
per instruction = 2x PE throughput); scores and the FFN run in bf16.
q/k/v and attention probabilities never leave SBUF. Scores are computed
k-major (out[k, q]) so no PE transposes are needed; the softmax
denominator comes from an fp8 ones-matmul and normalization is folded
into the attn output copy.

The softmax (DVE add + ACT exp, ~600ns per [128,512] tile each) is the
attention-phase bottleneck, so FFN chunk sc is emitted interleaved with
attention chunk qg=sc+2: the in-order PE queue then always has FFN
matmuls to run while the softmax chain drains. Cross-core comms: 4
chunked AllReduces of the attention output and 4 chunked ReduceScatters
of (attention + FFN) partials.
"""

import math

import numpy as np

S = 2048
D = 4096
H = 32
DH = 128
FF = 10240
NCORES = 8
P = 128
HPC = H // NCORES            # 4 heads per core
WPC = HPC * DH               # 512   per-core qkv width
FPC = FF // NCORES           # 1280  per-core ff width
FCC = FPC // P               # 10
DC = D // P                  # 32
DCH = DC // 2                # 16
SCN = 4                      # S chunks
SCW = S // SCN               # 512
KC = S // P                  # 16 key chunks
EPS = 1e-6

# fp8 weight scales (powers of two; descaled at psum copy-out)
S_WQ = 256.0                 # wq folded with 1/sqrt(DH): std ~0.0014
S_WK = 16.0
S_WV = 16.0
S_WO = 16.0
S_QS = 4.0                   # q stored as 4*q (e4m3); pb pre-scaled by 4 on host
S_VS = 8.0                   # v stored as 8*v; cancels with attn fp8 scale

_CACHE = {}


def _build():
    import concourse.bacc as bacc
    import concourse.tile as tile
    from concourse import mybir

    f32 = mybir.dt.float32
    bf = mybir.dt.bfloat16
    e4 = mybir.dt.float8e4
    e5 = mybir.dt.float8e5
    AF = mybir.ActivationFunctionType
    ALU = mybir.AluOpType
    DR = mybir.MatmulPerfMode.DoubleRow
    RG = [list(range(NCORES))]

    nc = bacc.Bacc(None, num_devices=NCORES)

    hT = nc.dram_tensor("hT", [DC, P, S], bf, kind="ExternalInput")
    h_own = nc.dram_tensor("h_own", [4, P, S], f32, kind="ExternalInput")
    wq = nc.dram_tensor("wq", [P, DCH, 2, WPC], e4, kind="ExternalInput")
    wk = nc.dram_tensor("wk", [P, DCH, 2, WPC], e4, kind="ExternalInput")
    wv = nc.dram_tensor("wv", [P, DCH, 2, WPC], e4, kind="ExternalInput")
    wo = nc.dram_tensor("wo", [P, 2, 2, D], e4, kind="ExternalInput")
    pbT = nc.dram_tensor("pbT", [HPC, SCN, 4, P, 4, SCW], bf, kind="ExternalInput")
    w01 = nc.dram_tensor("w01", [FCC, 2, P, 2, DCH, P], bf, kind="ExternalInput")
    wout = nc.dram_tensor("wout", [8, P, FCC, 4 * P], bf, kind="ExternalInput")
    ones4 = nc.dram_tensor("ones4", [P, 2, P], e4, kind="ExternalInput")
    ones5 = nc.dram_tensor("ones5", [P, 2, P], e5, kind="ExternalInput")
    out = nc.dram_tensor("out", [WPC, S], f32, kind="ExternalOutput")

    from contextlib import ExitStack

    with tile.TileContext(nc) as tc:
        with ExitStack() as ctx:
            ep = ctx.enter_context
            dram = ep(tc.tile_pool(name="dram", bufs=1, space="DRAM"))
            singles = ep(tc.tile_pool(name="singles", bufs=1))
            arena = ep(tc.tile_pool(name="arena", bufs=1))
            hstr = ep(tc.tile_pool(name="hstr", bufs=6))
            xarena = ep(tc.tile_pool(name="xarena", bufs=2))
            wstr = ep(tc.tile_pool(name="wstr", bufs=2))
            wff = ep(tc.tile_pool(name="wff", bufs=2))
            pbp = ep(tc.tile_pool(name="pbp", bufs=4))
            p5p = ep(tc.tile_pool(name="p5p", bufs=2))
            atp = ep(tc.tile_pool(name="atp", bufs=1))
            sap = ep(tc.tile_pool(name="sap", bufs=4))
            rbp = ep(tc.tile_pool(name="rbp", bufs=2))
            psA = ep(tc.tile_pool(name="psA", bufs=3, space="PSUM"))
            psB = ep(tc.tile_pool(name="psB", bufs=2, space="PSUM"))
            psF = ep(tc.tile_pool(name="psF", bufs=3, space="PSUM"))

            # ---- DRAM scratch for collectives ----
            arin = [dram.tile([DC, P, SCW], bf, tag=f"arin{j}", name=f"arin{j}")
                    for j in range(SCN)]
            arout = [dram.tile([DC, P, SCW], bf, tag=f"arout{j}", name=f"arout{j}",
                               addr_space="Shared") for j in range(SCN)]
            rsin = [dram.tile([DC, P, SCW], bf, tag=f"rsin{j}", name=f"rsin{j}")
                    for j in range(SCN)]
            rsout = [dram.tile([4, P, SCW], bf, tag=f"rsout{j}", name=f"rsout{j}")
                     for j in range(SCN)]

            ones4_sb = singles.tile([P, 2, P], e4)
            nc.sync.dma_start(out=ones4_sb[:], in_=ones4[:])
            ones5_sb = singles.tile([P, 2, P], e5)
            nc.sync.dma_start(out=ones5_sb[:], in_=ones5[:])
            eps_t = singles.tile([P, 1], f32)
            nc.vector.memset(eps_t[:], EPS)

            # persistent SBUF arenas for q/k/v (fp8)
            qT = arena.tile([P, HPC, S], e4, tag="qT")       # [dh, h, s] = 4*q
            kT = arena.tile([P, HPC, S], e4, tag="kT")       # [dh, h, s] = k
            v8 = arena.tile([P, HPC, 8, 2, DH], e4, tag="v8")  # 8*v

            # ================= phase 1: rmsnorm1 + QKV =====================
            def qkv_chunk(j):
                cols = slice(j * SCW, (j + 1) * SCW)
                quarters = []
                ss = psB.tile([P, SCW], f32, tag="pB", name="ss_ps")
                for qr in range(4):
                    hld = hstr.tile([P, 8, SCW], bf, tag="hstream",
                                    name=f"hld{qr}")
                    nc.sync.dma_start(
                        out=hld[:],
                        in_=hT[qr * 8:(qr + 1) * 8, :, cols].rearrange(
                            "d p s -> p d s"),
                    )
                    quarters.append(hld)
                    sq8 = p5p.tile([P, 8, SCW], e4, tag="sq8", name=f"sq8{qr}",
                                   bufs=1)
                    nc.vector.tensor_mul(sq8[:], hld[:], hld[:])
                    for jj in range(4):
                        nc.tensor.matmul(
                            ss[:], lhsT=ones4_sb[:],
                            rhs=sq8[:, 2 * jj:2 * jj + 2, :],
                            start=(qr == 0 and jj == 0),
                            stop=(qr == 3 and jj == 3),
                            perf_mode=DR,
                        )
                rbc = rbp.tile([P, SCW], f32, tag="rbc")
                nc.scalar.activation(
                    out=rbc[:], in_=ss[:], func=AF.Sqrt, bias=eps_t[:],
                    scale=1.0 / D,
                )
                nc.vector.reciprocal(out=rbc[:], in_=rbc[:])
                x8 = xarena.tile([P, DC, SCW], e4, tag="x8")
                for d in range(DC):
                    nc.vector.tensor_mul(
                        x8[:, d, :], quarters[d // 8][:, d % 8, :], rbc[:])

                for name, wsrc, dst, cscale in (
                    ("q", wq, qT, S_QS / S_WQ), ("k", wk, kT, 1.0 / S_WK),
                ):
                    wsb = wstr.tile([P, DCH, 2, WPC], e4, tag="wstream",
                                    name=f"w{name}sb")
                    nc.sync.dma_start(out=wsb[:], in_=wsrc[:])
                    for h in range(HPC):
                        ps = psA.tile([P, SCW], f32, tag="pA",
                                      name=f"ps_{name}{h}")
                        for dp in range(DCH):
                            nc.tensor.matmul(
                                ps[:], lhsT=wsb[:, dp, :, h * DH:(h + 1) * DH],
                                rhs=x8[:, 2 * dp:2 * dp + 2, :],
                                start=(dp == 0), stop=(dp == DCH - 1),
                                perf_mode=DR,
                            )
                        nc.scalar.mul(dst[:, h, cols], ps[:], cscale)

                wvsb = wstr.tile([P, DCH, 2, WPC], e4, tag="wstream", name="wvsb")
                nc.sync.dma_start(out=wvsb[:], in_=wv[:])
                for sl in range(SCW // P):
                    ps = psA.tile([P, WPC], f32, tag="pA", name=f"ps_v{sl}")
                    for dp in range(DCH):
                        nc.tensor.matmul(
                            ps[:], lhsT=x8[:, 2 * dp:2 * dp + 2, sl * P:(sl + 1) * P],
                            rhs=wvsb[:, dp, :, :],
                            start=(dp == 0), stop=(dp == DCH - 1),
                            perf_mode=DR,
                        )
                    kcix = j * (SCW // P) + sl
                    nc.scalar.mul(
                        v8[:, :, kcix // 2, kcix % 2, :],
                        ps[:].rearrange("p (h f) -> p h f", h=HPC),
                        S_VS / S_WV,
                    )

            # ================= attention units =============================
            def attn_scores(qg, h):
                qcols = slice(qg * SCW, (qg + 1) * SCW)
                p5 = p5p.tile([P, KC, SCW], e5, tag="p5", name="p5")
                for qtr in range(4):
                    pbt = pbp.tile([P, 4, SCW], bf, tag="pbt", name="pbt")
                    nc.sync.dma_start(out=pbt[:], in_=pbT[h, qg, qtr])
                    for kk in range(4):
                        kc = qtr * 4 + kk
                        pss = psA.tile([P, SCW], f32, tag="pA", name="pss")
                        nc.tensor.matmul(
                            pss[:], lhsT=kT[:, h, kc * P:(kc + 1) * P],
                            rhs=qT[:, h, qcols], start=True, stop=True,
                        )
                        sadd = sap.tile([P, SCW], bf, tag="sadd", bufs=2)
                        nc.vector.tensor_add(sadd[:], pss[:], pbt[:, kk, :])
                        nc.scalar.activation(
                            out=p5[:, kc, :], in_=sadd[:], func=AF.Exp,
                            scale=1.0 / S_QS,
                        )
                return p5

            def attn_sums_av(qg, h, p5, attnT):
                sums = psB.tile([P, SCW], f32, tag="pB", name="sums_ps")
                for jj in range(KC // 2):
                    nc.tensor.matmul(
                        sums[:], lhsT=ones5_sb[:],
                        rhs=p5[:, 2 * jj:2 * jj + 2, :],
                        start=(jj == 0), stop=(jj == KC // 2 - 1),
                        perf_mode=DR,
                    )
                psav = psB.tile([P, SCW], f32, tag="pB", name="psav")
                for jj in range(KC // 2):
                    nc.tensor.matmul(
                        psav[:], lhsT=v8[:, h, jj, :, :],
                        rhs=p5[:, 2 * jj:2 * jj + 2, :],
                        start=(jj == 0), stop=(jj == KC // 2 - 1),
                        perf_mode=DR,
                    )
                rs = rbp.tile([P, SCW], f32, tag="rbc", name="rs")
                nc.vector.reciprocal(out=rs[:], in_=sums[:])
                nc.vector.tensor_mul(attnT[:, h, :], psav[:], rs[:])

            def wo_unit(qg, attnT):
                wosb = wstr.tile([P, 2, 2, D], e4, tag="wstream", name="wosb")
                nc.sync.dma_start(out=wosb[:], in_=wo[:])
                for dg in range(8):
                    stg = p5p.tile([P, 4, SCW], bf, tag="stg", name="wostg",
                                   bufs=2)
                    for di in range(4):
                        dcc = dg * 4 + di
                        ps = psA.tile([P, SCW], f32, tag="pA", name="ps_wo")
                        for hp in range(2):
                            nc.tensor.matmul(
                                ps[:],
                                lhsT=wosb[:, hp, :, dcc * P:(dcc + 1) * P],
                                rhs=attnT[:, 2 * hp:2 * hp + 2, :],
                                start=(hp == 0), stop=(hp == 1),
                                perf_mode=DR,
                            )
                        nc.scalar.mul(stg[:, di, :], ps[:], 1.0 / (S_VS * S_WO))
                    nc.sync.dma_start(
                        out=arin[qg][dg * 4:(dg + 1) * 4, :, :].rearrange(
                            "d p s -> p d s"),
                        in_=stg[:],
                    )
                nc.gpsimd.collective_compute(
                    "AllReduce", ALU.add, replica_groups=RG,
                    ins=[arin[qg][:]], outs=[arout[qg][:]],
                )

            def attn_units(qg):
                """List of emit-callbacks for one attention chunk."""
                attnT = atp.tile([P, HPC, SCW], e4, tag="attnT",
                                 name=f"attnT{qg}")
                state = {}

                def u_scores(h):
                    def emit():
                        p5_prev = state.get("p5")
                        state["p5"] = attn_scores(qg, h)
                        if p5_prev is not None:
                            attn_sums_av(qg, h - 1, p5_prev, attnT)
                    return emit

                def u_tail():
                    def emit():
                        attn_sums_av(qg, HPC - 1, state["p5"], attnT)
                        wo_unit(qg, attnT)
                    return emit

                return [u_scores(h) for h in range(HPC)] + [u_tail()]

            # ================= FFN units ===================================
            def ffn_prologue(sc):
                cols = slice(sc * SCW, (sc + 1) * SCW)
                quarters = []
                ss2 = psB.tile([P, SCW], f32, tag="pB", name="ss2_ps")
                for qr in range(4):
                    h1h = hstr.tile([P, 8, SCW], bf, tag="hstream",
                                    name=f"h1h{qr}")
                    nc.sync.dma_start(
                        out=h1h[:],
                        in_=hT[qr * 8:(qr + 1) * 8, :, cols].rearrange(
                            "d p s -> p d s"),
                    )
                    for hv in range(2):
                        ars = pbp.tile([P, 4, SCW], bf, tag="pbt", name="ars")
                        d0 = qr * 8 + hv * 4
                        nc.sync.dma_start(
                            out=ars[:],
                            in_=arout[sc][d0:d0 + 4, :, :].rearrange(
                                "d p s -> p d s"),
                        )
                        nc.vector.tensor_add(
                            h1h[:, hv * 4:(hv + 1) * 4, :],
                            h1h[:, hv * 4:(hv + 1) * 4, :], ars[:])
                    quarters.append(h1h)
                    sq8 = p5p.tile([P, 8, SCW], e4, tag="sq8", name=f"fsq8{qr}",
                                   bufs=1)
                    nc.vector.tensor_mul(sq8[:], h1h[:], h1h[:])
                    for jj in range(4):
                        nc.tensor.matmul(
                            ss2[:], lhsT=ones4_sb[:],
                            rhs=sq8[:, 2 * jj:2 * jj + 2, :],
                            start=(qr == 0 and jj == 0),
                            stop=(qr == 3 and jj == 3),
                            perf_mode=DR,
                        )
                rbc2 = rbp.tile([P, SCW], f32, tag="rbc", name="rbc2")
                nc.scalar.activation(
                    out=rbc2[:], in_=ss2[:], func=AF.Sqrt, bias=eps_t[:],
                    scale=1.0 / D,
                )
                nc.vector.reciprocal(out=rbc2[:], in_=rbc2[:])
                for d in range(DC):
                    y = quarters[d // 8][:, d % 8, :]
                    nc.vector.tensor_mul(y, y, rbc2[:])
                ffT = xarena.tile([P, FCC, SCW], bf, tag="x8", name="ffT")
                return quarters, ffT

            def ffn_fc(sc, fc, quarters, ffT):
                w01h = [wff.tile([P, 2, DCH, P], bf, tag="wff",
                                 name=f"w01h{hh}") for hh in range(2)]
                for hh in range(2):
                    nc.sync.dma_start(out=w01h[hh][:], in_=w01[fc, hh])
                psg = psF.tile([P, SCW], f32, tag="pF", name="psg")
                psu = psF.tile([P, SCW], f32, tag="pF", name="psu")
                for d in range(DC):
                    y = quarters[d // 8][:, d % 8, :]
                    wt = w01h[d // DCH]
                    nc.tensor.matmul(
                        psg[:], lhsT=wt[:, 0, d % DCH, :], rhs=y,
                        start=(d == 0), stop=(d == DC - 1),
                    )
                    nc.tensor.matmul(
                        psu[:], lhsT=wt[:, 1, d % DCH, :], rhs=y,
                        start=(d == 0), stop=(d == DC - 1),
                    )
                gel = sap.tile([P, SCW], bf, tag="gel", bufs=1)
                nc.scalar.activation(out=gel[:], in_=psg[:], func=AF.Gelu)
                nc.vector.tensor_mul(ffT[:, fc, :], psu[:], gel[:])

            def ffn_wout_dg(sc, dg, ffT):
                arp = pbp.tile([P, 4, SCW], bf, tag="pbt", name="arp")
                nc.sync.dma_start(
                    out=arp[:],
                    in_=arin[sc][dg * 4:(dg + 1) * 4, :, :].rearrange(
                        "d p s -> p d s"),
                )
                wob = wstr.tile([P, FCC, 4 * P], bf, tag="wstream", name="wob")
                nc.sync.dma_start(out=wob[:], in_=wout[dg])
                stg = p5p.tile([P, 4, SCW], bf, tag="stg", name="ffstg", bufs=2)
                for di in range(4):
                    ps = psA.tile([P, SCW], f32, tag="pA", name="ps_o")
                    for fc in range(FCC):
                        nc.tensor.matmul(
                            ps[:], lhsT=wob[:, fc, di * P:(di + 1) * P],
                            rhs=ffT[:, fc, :],
                            start=(fc == 0), stop=(fc == FCC - 1),
                        )
                    nc.vector.tensor_add(stg[:, di, :], ps[:], arp[:, di, :])
                nc.sync.dma_start(
                    out=rsin[sc][dg * 4:(dg + 1) * 4, :, :].rearrange(
                        "d p s -> p d s"),
                    in_=stg[:],
                )

            def ffn_units(sc):
                state = {}

                def u_pro():
                    def emit():
                        state["q"], state["ffT"] = ffn_prologue(sc)
                    return emit

                def u_fc(fc):
                    def emit():
                        ffn_fc(sc, fc, state["q"], state["ffT"])
                    return emit

                def u_dg(dg):
                    def emit():
                        ffn_wout_dg(sc, dg, state["ffT"])
                        if dg == 7:
                            nc.gpsimd.collective_compute(
                                "ReduceScatter", ALU.add, replica_groups=RG,
                                ins=[rsin[sc][:]], outs=[rsout[sc][:]],
                            )
                    return emit

                return ([u_pro()] + [u_fc(fc) for fc in range(FCC)]
                        + [u_dg(dg) for dg in range(8)])

            def residual(sc):
                cols = slice(sc * SCW, (sc + 1) * SCW)
                rst = pbp.tile([P, 4, SCW], bf, tag="pbt", name="rst")
                nc.sync.dma_start(
                    out=rst[:], in_=rsout[sc][:].rearrange("o p s -> p o s"))
                hot = hstr.tile([P, 4, SCW], f32, tag="hstream", name="hot")
                nc.sync.dma_start(
                    out=hot[:], in_=h_own[:, :, cols].rearrange("o p s -> p o s"))
                for hv in range(2):
                    ot = p5p.tile([P, 2, SCW], f32, tag="stg", name="ot", bufs=2)
                    nc.vector.tensor_add(
                        ot[:], hot[:, 2 * hv:2 * hv + 2, :],
                        rst[:, 2 * hv:2 * hv + 2, :])
                    nc.sync.dma_start(
                        out=out[2 * hv * P:(2 * hv + 2) * P, cols].rearrange(
                            "(o p) s -> p o s", p=P),
                        in_=ot[:],
                    )

            # ================= emission schedule ===========================
            for j in range(SCN):
                qkv_chunk(j)

            # qg0, qg1: attention alone (FFN inputs not ready yet)
            for u in attn_units(0):
                u()
            for u in attn_units(1):
                u()
            # qg2 + FFN(0), qg3 + FFN(1): interleave (~1 attn : 3 ffn units)
            for qg in (2, 3):
                au = attn_units(qg)
                fu = ffn_units(qg - 2)
                sched = [fu[0]]
                fi = 1
                for i, a in enumerate(au):
                    sched.append(a)
                    take = 3 if i < len(au) - 1 else len(fu) - fi
                    for _ in range(min(take, len(fu) - fi)):
                        sched.append(fu[fi])
                        fi += 1
                while fi < len(fu):
                    sched.append(fu[fi])
                    fi += 1
                for u in sched:
                    u()
            residual(0)
            for sc in (2, 3):
                for u in ffn_units(sc):
                    u()
                residual(sc - 1)
            residual(3)

    nc.finalize()
    return nc


def _prep_in_maps(inputs):
    import ml_dtypes

    bf16 = ml_dtypes.bfloat16
    e4 = ml_dtypes.float8_e4m3
    e5 = ml_dtypes.float8_e5m2

    def q8(x, s):
        return np.clip(x * s, -240, 240).astype(e4)

    hid = np.ascontiguousarray(np.asarray(inputs["hidden_states"], np.float32)[0])
    mask = np.asarray(inputs["attention_mask"])[0]
    pbias = np.asarray(inputs["position_bias"], np.float32)[0]
    ln_a = np.asarray(inputs["ln_attn_w"], np.float32)
    ln_f = np.asarray(inputs["ln_ffn_w"], np.float32)
    wq = np.asarray(inputs["wq"], np.float32)
    wk = np.asarray(inputs["wk"], np.float32)
    wv = np.asarray(inputs["wv"], np.float32)
    wo = np.asarray(inputs["wo"], np.float32)
    w0 = np.asarray(inputs["w0"], np.float32)
    w1 = np.asarray(inputs["w1"], np.float32)
    w_out = np.asarray(inputs["w_out"], np.float32)

    hT = np.ascontiguousarray(hid.T)                          # (D, S) f32
    hT_bf = hT.reshape(DC, P, S).astype(bf16)
    wq_f = q8(ln_a[:, None] * wq * (DH ** -0.5), S_WQ)
    wk_f = q8(ln_a[:, None] * wk, S_WK)
    wv_f = q8(ln_a[:, None] * wv, S_WV)
    wo_f = q8(wo, S_WO)
    w0_f = (ln_f[:, None] * w0).astype(bf16)
    w1_f = (ln_f[:, None] * w1).astype(bf16)
    wout_f = w_out.astype(bf16)
    if mask.all():
        pb_m = pbias * S_QS
    else:
        pb_m = np.where(mask[None], pbias * S_QS, np.float32(-1e30))
    # transposed position bias: [H, S_k, S_q] -> per-core [HPC,SCN,4,P,4,SCW]
    pbT_full = np.ascontiguousarray(pb_m.transpose(0, 2, 1)).astype(bf16)

    ones4 = np.ones((P, 2, P), dtype=e4)
    ones5 = np.ones((P, 2, P), dtype=e5)

    def wqkv_layout(w):                # (D, WPC) -> (P, DCH, 2, WPC)
        return np.ascontiguousarray(
            w.reshape(DCH, 2, P, WPC).transpose(2, 0, 1, 3))

    in_maps = []
    for c in range(NCORES):
        ws = slice(c * WPC, (c + 1) * WPC)
        fs = slice(c * FPC, (c + 1) * FPC)
        wo_c = np.ascontiguousarray(
            wo_f[ws, :].reshape(2, 2, P, D).transpose(2, 0, 1, 3))
        pb_c = pbT_full[c * HPC:(c + 1) * HPC]                # (HPC, S_k, S_q)
        pb_c = pb_c.reshape(HPC, 4, 4, P, SCN, SCW).transpose(0, 4, 1, 3, 2, 5)
        w0_c = w0_f[:, fs].reshape(DC, P, FCC, P).transpose(2, 1, 0, 3)
        w1_c = w1_f[:, fs].reshape(DC, P, FCC, P).transpose(2, 1, 0, 3)
        w01_c = np.stack([w0_c, w1_c], axis=1)                # [FCC,2,P,DC,P]
        w01_c = w01_c.reshape(FCC, 2, P, 2, DCH, P).transpose(0, 3, 2, 1, 4, 5)
        wout_c = wout_f[fs, :].reshape(FCC, P, 8, 4 * P).transpose(2, 1, 0, 3)
        in_maps.append({
            "hT": hT_bf,
            "h_own": np.ascontiguousarray(hT[ws].reshape(4, P, S)),
            "wq": wqkv_layout(wq_f[:, ws]),
            "wk": wqkv_layout(wk_f[:, ws]),
            "wv": wqkv_layout(wv_f[:, ws]),
            "wo": wo_c,
            "pbT": np.ascontiguousarray(pb_c),
            "w01": np.ascontiguousarray(w01_c),
            "wout": np.ascontiguousarray(wout_c),
            "ones4": ones4,
            "ones5": ones5,
        })
    return in_maps


def get_nc():
    if "nc" not in _CACHE:
        _CACHE["nc"] = _build()
    return _CACHE["nc"]


def kernel(**inputs):
    from concourse.bass_utils import run_bass_kernel_spmd

    nc = get_nc()
    in_maps = _prep_in_maps(inputs)
    res = run_bass_kernel_spmd(nc, in_maps, core_ids=list(range(NCORES)))
    parts = [res.results[c]["out"] for c in range(NCORES)]   # each (WPC, S)
    full_T = np.concatenate(parts, axis=0)                    # (D, S)
    out = np.ascontiguousarray(full_T.T)[None]                # (1, S, D)
    return out.astype(np.float32)
